# revision 18
# baseline (speedup 1.0000x reference)
"""Bass/Trainium2 kernel for nn_BilinearInteractionLayer.

Computes, for all field pairs (i, j) with i < j (P = C(32,2) = 496 pairs):
    out[b, p, :] = (emb[b, i_p, :] @ W[p].T) * emb[b, j_p, :]
with emb [2048, 32, 64] fp32 and W [496, 64, 64] fp32.

Strategy: data-parallel over batch across 8 cores (B=256 per core), W
replicated. Per core, for each field f the pairs (f, j), j>f are contiguous in
the global pair order, so matmuls per (b-chunk, field) compute
proj[b, (j, e)] = X_f[b, :] @ Wcat_f.T with the batch chunk (128 rows) as the
PE stationary operand and the stacked pair weights streaming. Each PSUM chunk
is evicted by a single DVE tensor_mul against the naturally-laid-out
emb[b, f+1:, :] slice (fusing the v_j multiply), staged in SBUF, and DMAed to
HBM in large contiguous-per-partition transfers.

The 496 pairs are split into two "planes" (fields 0..8 -> SBUF partitions
0:64, fields 9..30 -> partitions 64:128) so the packed weight tensor uses all
128 partitions (full DMA port bandwidth) and the two planes' matmuls run
concurrently on distinct PE row-groups (K=64 each, tile_position (0,0) and
(64,0)); matmul instructions are interleaved per-MM across planes to realize
the overlap.

Matmul operands are cast to fp16 on the host (10-bit mantissa; rel err of the
result ~3e-4 with fp32 PSUM accumulation); the v_j factor is fp16 as well
(adds ~2e-4), while accumulation/eviction stays fp32.
"""

import sys

sys.path.insert(0, "/opt/trn_rl_repo")

from contextlib import ExitStack
from itertools import combinations

import numpy as np

import concourse.bass as bass
import concourse.tile as tile
from concourse import bacc, bass_utils, mybir
from concourse._compat import with_exitstack

NUM_FIELDS = 32
EMB_DIM = 64
BATCH = 2048
N_CORES = 8
B_CORE = BATCH // N_CORES          # 256
N_BCHUNK = B_CORE // 128           # 2
PAIRS = list(combinations(range(NUM_FIELDS), 2))
P_TOTAL = len(PAIRS)               # 496

# OFF[f] = global pair index of first pair (f, f+1)
OFF = [0] * NUM_FIELDS
for _f in range(1, NUM_FIELDS):
    OFF[_f] = OFF[_f - 1] + (NUM_FIELDS - _f)

# Plane split: fields 0..8 (243 pairs) on partitions 0:64, fields 9..30
# (253 pairs) on partitions 64:128.
PLANE_FIELDS = (list(range(0, 9)), list(range(9, 31)))
PLANE_P0 = (0, OFF[9])                       # 0, 243
PLANE_NP = (OFF[9] - 0, P_TOTAL - OFF[9])    # 243, 253
WT_COLS = max(PLANE_NP) * EMB_DIM            # 16192

MM_N = 512            # max moving free dim per matmul (one PSUM bank, fp32)
PSUM_COLS = 1024      # psum tile width (2 banks)
STAGE_COLS = 4096     # stage tile width
WT_CHUNK = 4096       # wt DMA chunk (pair-aligned: 64 pairs)
WT_NCHUNK = (WT_COLS + WT_CHUNK - 1) // WT_CHUNK  # 4
EMBN_DT = mybir.dt.float16
EMBN_NP = np.float16
# Output is written to HBM as fp16 (halving the dominant DMA stream) and
# upcast to fp32 on the host during the gather.
OUT_DT = mybir.dt.float16
OUT_NP = np.float16


def _field_groups(plane):
    """Group consecutive fields of a plane so each group's output columns fit
    in one stage tile (one output DMA per group per b-chunk)."""
    groups = []
    cur, cur_cols = [], 0
    for f in PLANE_FIELDS[plane]:
        cols = (NUM_FIELDS - 1 - f) * EMB_DIM
        if cur and cur_cols + cols > STAGE_COLS:
            groups.append(cur)
            cur, cur_cols = [], 0
        cur.append(f)
        cur_cols += cols
    if cur:
        groups.append(cur)
    return groups


def _plane_entries(plane):
    """Flatten a plane's work into psum-chunk entries, in program order.

    Entry: dict(plane, c, f, group_key, stage_off, chunk0, cols, mms, first_in_group,
    last_in_group, group_pair0, group_npairs)."""
    entries = []
    groups = _field_groups(plane)
    for c in range(N_BCHUNK):
        for gi, fields in enumerate(groups):
            group_pair0 = OFF[fields[0]]
            group_npairs = sum(NUM_FIELDS - 1 - f for f in fields)
            stage_off = 0
            for fi, f in enumerate(fields):
                cols = (NUM_FIELDS - 1 - f) * EMB_DIM
                p_local = OFF[f] - PLANE_P0[plane]
                col0 = p_local * EMB_DIM
                for ck0 in range(0, cols, PSUM_COLS):
                    ccols = min(PSUM_COLS, cols - ck0)
                    # split into matmuls at MM_N and wt-chunk boundaries
                    mms = []
                    k0 = 0
                    while k0 < ccols:
                        abs_col = col0 + ck0 + k0
                        n = min(MM_N, ccols - k0)
                        # don't cross a wt DMA-chunk boundary (separate tiles)
                        chunk_end = ((abs_col // WT_CHUNK) + 1) * WT_CHUNK
                        n = min(n, chunk_end - abs_col)
                        # don't cross a PSUM bank boundary (512 fp32 cols)
                        n = min(n, MM_N - (k0 % MM_N))
                        mms.append((abs_col, ck0 + k0, n))
                        k0 += n
                    entries.append(
                        dict(
                            plane=plane,
                            c=c,
                            f=f,
                            group_key=(plane, c, gi),
                            stage_off=stage_off + ck0,
                            cols=ccols,
                            ck0=ck0,
                            mms=mms,
                            first_in_group=(fi == 0 and ck0 == 0),
                            last_in_group=(
                                fi == len(fields) - 1 and ck0 + ccols >= cols
                            ),
                            group_pair0=group_pair0,
                            group_npairs=group_npairs,
                        )
                    )
                stage_off += cols
    return entries


@with_exitstack
def _bilinear_kernel(
    ctx: ExitStack,
    tc: "tile.TileContext",
    out_ap: bass.AP,
    wt_aps,
    embt_aps,
    embn_aps,
):
    nc = tc.nc

    wt_pool = ctx.enter_context(tc.tile_pool(name="wt", bufs=WT_NCHUNK))
    embt_pool = ctx.enter_context(tc.tile_pool(name="embt", bufs=N_BCHUNK))
    embn_pool = ctx.enter_context(tc.tile_pool(name="embn", bufs=N_BCHUNK))
    psum_pool = ctx.enter_context(tc.tile_pool(name="psum", bufs=4, space="PSUM"))
    stage_pool = ctx.enter_context(tc.tile_pool(name="stage", bufs=5))

    # Input loads split across both HWDGE rings so they issue in parallel;
    # b-chunk 0 data lands first so compute starts early. Output stores share
    # the SP ring but only begin after the inputs have drained.
    embt_lo, embn_tiles = [], []
    for c in range(N_BCHUNK):
        lo = embt_pool.tile(
            [128, NUM_FIELDS * 128], mybir.dt.float16, tag="embtl", name=f"embtl{c}"
        )
        embt_lo.append(lo)
        en = embn_pool.tile(
            [128, NUM_FIELDS * EMB_DIM], EMBN_DT, tag="embn", name=f"embn{c}"
        )
        embn_tiles.append(en)
    wt_tiles = []
    for k in range(WT_NCHUNK):
        cols = min(WT_CHUNK, WT_COLS - k * WT_CHUNK)
        t = wt_pool.tile([128, cols], mybir.dt.float16, tag="wt", name=f"wtt{k}")
        wt_tiles.append(t)
    nc.sync.dma_start(embt_lo[0][0:64, :], embt_aps[0][:])
    nc.scalar.dma_start(wt_tiles[0][:], wt_aps[0][:])
    nc.sync.dma_start(embn_tiles[0][:], embn_aps[0][:])
    nc.gpsimd.dma_start(embt_lo[0][64:128, :], embt_lo[0][0:64, :])
    for k in range(1, WT_NCHUNK):
        nc.scalar.dma_start(wt_tiles[k][:], wt_aps[k][:])
    nc.scalar.dma_start(embt_lo[1][0:64, :], embt_aps[1][:])
    nc.gpsimd.dma_start(embt_lo[1][64:128, :], embt_lo[1][0:64, :])
    nc.scalar.dma_start(embn_tiles[1][:], embn_aps[1][:])

    queues = [_plane_entries(0), _plane_entries(1)]
    idx = [0, 0]
    stages = {}  # group_key -> stage tile

    out_count = [0]

    def emit(entry_list):
        # allocate psum + stage, emit interleaved matmuls, then TT + DMA
        for e in entry_list:
            if e["first_in_group"]:
                stages[e["group_key"]] = stage_pool.tile(
                    [128, STAGE_COLS], OUT_DT, tag="stage", name="stg"
                )
            e["ps"] = psum_pool.tile([128, PSUM_COLS], mybir.dt.float32, tag="ps", name="ps")
            r0 = 64 * e["plane"]
            e["lhsT"] = embt_lo[e["c"]][
                r0 : r0 + 64, e["f"] * 128 : e["f"] * 128 + 128
            ]
        # zip matmuls across entries (planes) for PE row-group overlap
        maxmm = max(len(e["mms"]) for e in entry_list)
        for k in range(maxmm):
            for e in entry_list:
                if k < len(e["mms"]):
                    abs_col, pk0, n = e["mms"][k]
                    r0 = 64 * e["plane"]
                    wtt = wt_tiles[abs_col // WT_CHUNK]
                    wc = abs_col % WT_CHUNK
                    nc.tensor.matmul(
                        e["ps"][:, pk0 - e["ck0"] : pk0 - e["ck0"] + n],
                        e["lhsT"],
                        wtt[r0 : r0 + 64, wc : wc + n],
                        start=True,
                        stop=True,
                    )
        for e in entry_list:
            st = stages[e["group_key"]]
            e0 = (e["f"] + 1) * EMB_DIM + e["ck0"]
            dst = st[:, e["stage_off"] : e["stage_off"] + e["cols"]]
            in1 = embn_tiles[e["c"]][:, e0 : e0 + e["cols"]]
            # Alternate eviction paths to split work between DVE and ACT:
            #  A (1/3): DVE tensor_mul straight from PSUM (fp32 src, 1x mode)
            #  B (2/3): ACT copies PSUM->SBUF fp16, then the DVE multiply is
            #           all-16-bit SBUF->SBUF and runs in 2x_1P mode.
            if out_count[0] % 3 == 0:
                nc.vector.tensor_mul(dst, e["ps"][:, 0 : e["cols"]], in1)
            else:
                tmp = evict_pool.tile(
                    [128, PSUM_COLS], mybir.dt.float16, tag="ev", name="ev"
                )
                nc.scalar.copy(tmp[:, 0 : e["cols"]], e["ps"][:, 0 : e["cols"]])
                nc.vector.tensor_mul(dst, tmp[:, 0 : e["cols"]], in1)
            out_count[0] += 1
            if e["last_in_group"]:
                c, p0, npair = e["c"], e["group_pair0"], e["group_npairs"]
                nc.sync.dma_start(
                    out_ap[c * 128 : (c + 1) * 128, p0 : p0 + npair, :],
                    st[:, 0 : npair * EMB_DIM],
                )
                del stages[e["group_key"]]

    while idx[0] < len(queues[0]) or idx[1] < len(queues[1]):
        batch = []
        for p in (0, 1):
            if idx[p] < len(queues[p]):
                batch.append(queues[p][idx[p]])
                idx[p] += 1
        emit(batch)


_CACHE = {}


def _get_program():
    if "nc" not in _CACHE:
        nc = bacc.Bacc(
            "TRN2", target_bir_lowering=False, debug=False, num_devices=N_CORES
        )
        wt_aps = []
        for k in range(WT_NCHUNK):
            cols = min(WT_CHUNK, WT_COLS - k * WT_CHUNK)
            wt_aps.append(
                nc.dram_tensor(
                    f"wt{k}", [128, cols], mybir.dt.float16, kind="ExternalInput"
                ).ap()
            )
        embt_aps = [
            nc.dram_tensor(
                f"embt{c}", [64, NUM_FIELDS * 128], mybir.dt.float16,
                kind="ExternalInput",
            ).ap()
            for c in range(N_BCHUNK)
        ]
        embn_aps = [
            nc.dram_tensor(
                f"embn{c}", [128, NUM_FIELDS * EMB_DIM], EMBN_DT,
                kind="ExternalInput",
            ).ap()
            for c in range(N_BCHUNK)
        ]
        out_ap = nc.dram_tensor(
            "out", [B_CORE, P_TOTAL, EMB_DIM], OUT_DT, kind="ExternalOutput"
        ).ap()
        with tile.TileContext(nc) as tc:
            _bilinear_kernel(tc, out_ap, wt_aps, embt_aps, embn_aps)
        nc.compile()
        _CACHE["nc"] = nc
    return _CACHE["nc"]


def _pack_wt(W: np.ndarray):
    """W [496, 64, 64] fp32 -> WT_NCHUNK chunks of [128, <=4096] fp16 with
    wt[64*plane + d, p_local*64 + e] = W[p, e, d]."""
    Wh = W.astype(np.float16)
    full = np.zeros((128, WT_COLS), dtype=np.float16)
    for plane in (0, 1):
        p0, npair = PLANE_P0[plane], PLANE_NP[plane]
        blk = Wh[p0 : p0 + npair].transpose(2, 0, 1).reshape(EMB_DIM, npair * EMB_DIM)
        full[64 * plane : 64 * plane + EMB_DIM, : npair * EMB_DIM] = blk
    return [
        np.ascontiguousarray(full[:, k * WT_CHUNK : min((k + 1) * WT_CHUNK, WT_COLS)])
        for k in range(WT_NCHUNK)
    ]


def _pack_core_inputs(emb_shard: np.ndarray):
    """emb_shard [256, 32, 64] fp32 -> per-b-chunk (embt [128, 4096] f16,
    embn [128, 2048]) arrays; embt col = f*128 + r, duplicated partition halves."""
    embts, embns = [], []
    for c in range(N_BCHUNK):
        chunk = emb_shard[c * 128 : (c + 1) * 128]  # [128, 32, 64]
        et = (
            chunk.transpose(2, 1, 0).reshape(EMB_DIM, NUM_FIELDS * 128)
        ).astype(np.float16)
        embts.append(np.ascontiguousarray(et))
        embns.append(
            np.ascontiguousarray(
                chunk.reshape(128, NUM_FIELDS * EMB_DIM).astype(EMBN_NP)
            )
        )
    return embts, embns


def build_in_maps(feature_emb: np.ndarray, W: np.ndarray):
    wt_chunks = _pack_wt(np.asarray(W))
    emb = np.asarray(feature_emb, dtype=np.float32)
    in_maps = []
    for i in range(N_CORES):
        embts, embns = _pack_core_inputs(emb[i * B_CORE : (i + 1) * B_CORE])
        m = {}
        for c in range(N_BCHUNK):
            m[f"embt{c}"] = embts[c]
            m[f"embn{c}"] = embns[c]
        for k, w in enumerate(wt_chunks):
            m[f"wt{k}"] = w
        in_maps.append(m)
    return in_maps


def run(feature_emb: np.ndarray, W: np.ndarray, trace: bool = False, tmpdir=None):
    """Returns (out [2048, 496, 64] fp32, BassKernelResults)."""
    nc = _get_program()
    in_maps = build_in_maps(feature_emb, W)
    res = bass_utils.run_bass_kernel_spmd(
        nc, in_maps, core_ids=list(range(N_CORES)), trace=trace, tmpdir=tmpdir
    )
    out = np.concatenate(
        [res.results[i]["out"] for i in range(N_CORES)], axis=0
    ).astype(np.float32)
    return out, res


def kernel(feature_emb: np.ndarray, W: np.ndarray) -> np.ndarray:
    out, _ = run(feature_emb, W)
    return out


# revision 20
# speedup vs baseline: 1.0842x; 1.0842x over previous
"""Bass/Trainium2 kernel for nn_BilinearInteractionLayer.

Computes, for all field pairs (i, j) with i < j (P = C(32,2) = 496 pairs):
    out[b, p, :] = (emb[b, i_p, :] @ W[p].T) * emb[b, j_p, :]
with emb [2048, 32, 64] fp32 and W [496, 64, 64] fp32.

Strategy: data-parallel over batch across 8 cores (B=256 per core), W
replicated. Per core, for each field f the pairs (f, j), j>f are contiguous in
the global pair order, so matmuls per (b-chunk, field) compute
proj[b, (j, e)] = X_f[b, :] @ Wcat_f.T with the batch chunk (128 rows) as the
PE stationary operand and the stacked pair weights streaming. Each PSUM chunk
is evicted by a single DVE tensor_mul against the naturally-laid-out
emb[b, f+1:, :] slice (fusing the v_j multiply), staged in SBUF, and DMAed to
HBM in large contiguous-per-partition transfers.

The 496 pairs are split into two "planes" (fields 0..8 -> SBUF partitions
0:64, fields 9..30 -> partitions 64:128) so the packed weight tensor uses all
128 partitions (full DMA port bandwidth) and the two planes' matmuls run
concurrently on distinct PE row-groups (K=64 each, tile_position (0,0) and
(64,0)); matmul instructions are interleaved per-MM across planes to realize
the overlap.

Matmul operands are cast to fp16 on the host (10-bit mantissa; rel err of the
result ~3e-4 with fp32 PSUM accumulation); the v_j factor is fp16 as well
(adds ~2e-4), while accumulation/eviction stays fp32.
"""

import sys

sys.path.insert(0, "/opt/trn_rl_repo")

from contextlib import ExitStack
from itertools import combinations

import numpy as np

import concourse.bass as bass
import concourse.tile as tile
from concourse import bacc, bass_utils, mybir
from concourse._compat import with_exitstack

NUM_FIELDS = 32
EMB_DIM = 64
BATCH = 2048
N_CORES = 8
B_CORE = BATCH // N_CORES          # 256
N_BCHUNK = B_CORE // 128           # 2
PAIRS = list(combinations(range(NUM_FIELDS), 2))
P_TOTAL = len(PAIRS)               # 496

# OFF[f] = global pair index of first pair (f, f+1)
OFF = [0] * NUM_FIELDS
for _f in range(1, NUM_FIELDS):
    OFF[_f] = OFF[_f - 1] + (NUM_FIELDS - _f)

# Plane split: fields 0..8 (243 pairs) on partitions 0:64, fields 9..30
# (253 pairs) on partitions 64:128.
PLANE_FIELDS = (list(range(0, 9)), list(range(9, 31)))
PLANE_P0 = (0, OFF[9])                       # 0, 243
PLANE_NP = (OFF[9] - 0, P_TOTAL - OFF[9])    # 243, 253
WT_COLS = max(PLANE_NP) * EMB_DIM            # 16192

MM_N = 512            # max moving free dim per matmul (one PSUM bank, fp32)
PSUM_COLS = 1024      # psum tile width (2 banks)
STAGE_COLS = 4096     # stage tile width
WT_CHUNK = 4096       # wt DMA chunk (pair-aligned: 64 pairs)
WT_NCHUNK = (WT_COLS + WT_CHUNK - 1) // WT_CHUNK  # 4
EMBN_DT = mybir.dt.float16
EMBN_NP = np.float16
# Output is written to HBM as fp16 (halving the dominant DMA stream) and
# upcast to fp32 on the host during the gather.
OUT_DT = mybir.dt.float16
OUT_NP = np.float16


def _field_groups(plane):
    """Group consecutive fields of a plane so each group's output columns fit
    in one stage tile (one output DMA per group per b-chunk)."""
    groups = []
    cur, cur_cols = [], 0
    for f in PLANE_FIELDS[plane]:
        cols = (NUM_FIELDS - 1 - f) * EMB_DIM
        if cur and cur_cols + cols > STAGE_COLS:
            groups.append(cur)
            cur, cur_cols = [], 0
        cur.append(f)
        cur_cols += cols
    if cur:
        groups.append(cur)
    return groups


def _plane_entries(plane):
    """Flatten a plane's work into psum-chunk entries, in program order.

    Entry: dict(plane, c, f, group_key, stage_off, chunk0, cols, mms, first_in_group,
    last_in_group, group_pair0, group_npairs)."""
    entries = []
    groups = _field_groups(plane)
    for c in range(N_BCHUNK):
        for gi, fields in enumerate(groups):
            group_pair0 = OFF[fields[0]]
            group_npairs = sum(NUM_FIELDS - 1 - f for f in fields)
            stage_off = 0
            for fi, f in enumerate(fields):
                cols = (NUM_FIELDS - 1 - f) * EMB_DIM
                p_local = OFF[f] - PLANE_P0[plane]
                col0 = p_local * EMB_DIM
                for ck0 in range(0, cols, PSUM_COLS):
                    ccols = min(PSUM_COLS, cols - ck0)
                    # split into matmuls at MM_N and wt-chunk boundaries
                    mms = []
                    k0 = 0
                    while k0 < ccols:
                        abs_col = col0 + ck0 + k0
                        n = min(MM_N, ccols - k0)
                        # don't cross a wt DMA-chunk boundary (separate tiles)
                        chunk_end = ((abs_col // WT_CHUNK) + 1) * WT_CHUNK
                        n = min(n, chunk_end - abs_col)
                        # don't cross a PSUM bank boundary (512 fp32 cols)
                        n = min(n, MM_N - (k0 % MM_N))
                        mms.append((abs_col, ck0 + k0, n))
                        k0 += n
                    entries.append(
                        dict(
                            plane=plane,
                            c=c,
                            f=f,
                            group_key=(plane, c, gi),
                            stage_off=stage_off + ck0,
                            cols=ccols,
                            ck0=ck0,
                            mms=mms,
                            first_in_group=(fi == 0 and ck0 == 0),
                            last_in_group=(
                                fi == len(fields) - 1 and ck0 + ccols >= cols
                            ),
                            group_pair0=group_pair0,
                            group_npairs=group_npairs,
                        )
                    )
                stage_off += cols
    return entries


@with_exitstack
def _bilinear_kernel(
    ctx: ExitStack,
    tc: "tile.TileContext",
    out_ap: bass.AP,
    wt_aps,
    embt_aps,
    embn_aps,
):
    nc = tc.nc

    wt_pool = ctx.enter_context(tc.tile_pool(name="wt", bufs=WT_NCHUNK))
    embt_pool = ctx.enter_context(tc.tile_pool(name="embt", bufs=N_BCHUNK))
    embn_pool = ctx.enter_context(tc.tile_pool(name="embn", bufs=N_BCHUNK))
    psum_pool = ctx.enter_context(tc.tile_pool(name="psum", bufs=4, space="PSUM"))
    stage_pool = ctx.enter_context(tc.tile_pool(name="stage", bufs=5))

    # Input loads split across both HWDGE rings so they issue in parallel;
    # b-chunk 0 data lands first so compute starts early. Output stores share
    # the SP ring but only begin after the inputs have drained.
    embt_lo, embn_tiles = [], []
    for c in range(N_BCHUNK):
        lo = embt_pool.tile(
            [128, NUM_FIELDS * 128], mybir.dt.float16, tag="embtl", name=f"embtl{c}"
        )
        embt_lo.append(lo)
        en = embn_pool.tile(
            [128, NUM_FIELDS * EMB_DIM], EMBN_DT, tag="embn", name=f"embn{c}"
        )
        embn_tiles.append(en)
    wt_tiles = []
    for k in range(WT_NCHUNK):
        cols = min(WT_CHUNK, WT_COLS - k * WT_CHUNK)
        t = wt_pool.tile([128, cols], mybir.dt.float16, tag="wt", name=f"wtt{k}")
        wt_tiles.append(t)
    nc.sync.dma_start(embt_lo[0][0:64, :], embt_aps[0][:])
    nc.scalar.dma_start(wt_tiles[0][:], wt_aps[0][:])
    nc.sync.dma_start(embn_tiles[0][:], embn_aps[0][:])
    nc.gpsimd.dma_start(embt_lo[0][64:128, :], embt_lo[0][0:64, :])
    for k in range(1, WT_NCHUNK):
        nc.scalar.dma_start(wt_tiles[k][:], wt_aps[k][:])
    nc.scalar.dma_start(embt_lo[1][0:64, :], embt_aps[1][:])
    nc.gpsimd.dma_start(embt_lo[1][64:128, :], embt_lo[1][0:64, :])
    nc.scalar.dma_start(embn_tiles[1][:], embn_aps[1][:])

    queues = [_plane_entries(0), _plane_entries(1)]
    idx = [0, 0]
    stages = {}  # group_key -> stage tile

    out_count = [0]

    def emit(entry_list):
        # allocate psum + stage, emit interleaved matmuls, then TT + DMA
        for e in entry_list:
            if e["first_in_group"]:
                stages[e["group_key"]] = stage_pool.tile(
                    [128, STAGE_COLS], OUT_DT, tag="stage", name="stg"
                )
            e["ps"] = psum_pool.tile([128, PSUM_COLS], mybir.dt.float32, tag="ps", name="ps")
            r0 = 64 * e["plane"]
            e["lhsT"] = embt_lo[e["c"]][
                r0 : r0 + 64, e["f"] * 128 : e["f"] * 128 + 128
            ]
        # zip matmuls across entries (planes) for PE row-group overlap
        maxmm = max(len(e["mms"]) for e in entry_list)
        for k in range(maxmm):
            for e in entry_list:
                if k < len(e["mms"]):
                    abs_col, pk0, n = e["mms"][k]
                    r0 = 64 * e["plane"]
                    wtt = wt_tiles[abs_col // WT_CHUNK]
                    wc = abs_col % WT_CHUNK
                    nc.tensor.matmul(
                        e["ps"][:, pk0 - e["ck0"] : pk0 - e["ck0"] + n],
                        e["lhsT"],
                        wtt[r0 : r0 + 64, wc : wc + n],
                        start=True,
                        stop=True,
                    )
        for e in entry_list:
            st = stages[e["group_key"]]
            e0 = (e["f"] + 1) * EMB_DIM + e["ck0"]
            dst = st[:, e["stage_off"] : e["stage_off"] + e["cols"]]
            in1 = embn_tiles[e["c"]][:, e0 : e0 + e["cols"]]
            # Alternate eviction paths to split work between DVE and ACT:
            #  A (1/3): DVE tensor_mul straight from PSUM (fp32 src, 1x mode)
            #  B (2/3): ACT copies PSUM->SBUF fp16, then the DVE multiply is
            #           all-16-bit SBUF->SBUF and runs in 2x_1P mode.
            if out_count[0] % 3 == 0:
                nc.vector.tensor_mul(dst, e["ps"][:, 0 : e["cols"]], in1)
            else:
                tmp = evict_pool.tile(
                    [128, PSUM_COLS], mybir.dt.float16, tag="ev", name="ev"
                )
                nc.scalar.copy(tmp[:, 0 : e["cols"]], e["ps"][:, 0 : e["cols"]])
                nc.vector.tensor_mul(dst, tmp[:, 0 : e["cols"]], in1)
            out_count[0] += 1
            if e["last_in_group"]:
                c, p0, npair = e["c"], e["group_pair0"], e["group_npairs"]
                nc.sync.dma_start(
                    out_ap[c * 128 : (c + 1) * 128, p0 : p0 + npair, :],
                    st[:, 0 : npair * EMB_DIM],
                )
                del stages[e["group_key"]]

    while idx[0] < len(queues[0]) or idx[1] < len(queues[1]):
        batch = []
        for p in (0, 1):
            if idx[p] < len(queues[p]):
                batch.append(queues[p][idx[p]])
                idx[p] += 1
        emit(batch)


_CACHE = {}


def _get_program():
    if "nc" not in _CACHE:
        nc = bacc.Bacc(
            "TRN2", target_bir_lowering=False, debug=False, num_devices=N_CORES
        )
        wt_aps = []
        for k in range(WT_NCHUNK):
            cols = min(WT_CHUNK, WT_COLS - k * WT_CHUNK)
            wt_aps.append(
                nc.dram_tensor(
                    f"wt{k}", [128, cols], mybir.dt.float16, kind="ExternalInput"
                ).ap()
            )
        embt_aps = [
            nc.dram_tensor(
                f"embt{c}", [64, NUM_FIELDS * 128], mybir.dt.float16,
                kind="ExternalInput",
            ).ap()
            for c in range(N_BCHUNK)
        ]
        embn_aps = [
            nc.dram_tensor(
                f"embn{c}", [128, NUM_FIELDS * EMB_DIM], EMBN_DT,
                kind="ExternalInput",
            ).ap()
            for c in range(N_BCHUNK)
        ]
        out_ap = nc.dram_tensor(
            "out", [B_CORE, P_TOTAL, EMB_DIM], OUT_DT, kind="ExternalOutput"
        ).ap()
        with tile.TileContext(nc) as tc:
            _bilinear_kernel(tc, out_ap, wt_aps, embt_aps, embn_aps)
        nc.compile()
        _CACHE["nc"] = nc
    return _CACHE["nc"]


def _pack_wt(W: np.ndarray):
    """W [496, 64, 64] fp32 -> WT_NCHUNK chunks of [128, <=4096] fp16 with
    wt[64*plane + d, p_local*64 + e] = W[p, e, d]."""
    Wh = W.astype(np.float16)
    full = np.zeros((128, WT_COLS), dtype=np.float16)
    for plane in (0, 1):
        p0, npair = PLANE_P0[plane], PLANE_NP[plane]
        blk = Wh[p0 : p0 + npair].transpose(2, 0, 1).reshape(EMB_DIM, npair * EMB_DIM)
        full[64 * plane : 64 * plane + EMB_DIM, : npair * EMB_DIM] = blk
    return [
        np.ascontiguousarray(full[:, k * WT_CHUNK : min((k + 1) * WT_CHUNK, WT_COLS)])
        for k in range(WT_NCHUNK)
    ]


def _pack_core_inputs(emb_shard: np.ndarray):
    """emb_shard [256, 32, 64] fp32 -> per-b-chunk (embt [128, 4096] f16,
    embn [128, 2048]) arrays; embt col = f*128 + r, duplicated partition halves."""
    embts, embns = [], []
    for c in range(N_BCHUNK):
        chunk = emb_shard[c * 128 : (c + 1) * 128]  # [128, 32, 64]
        et = (
            chunk.transpose(2, 1, 0).reshape(EMB_DIM, NUM_FIELDS * 128)
        ).astype(np.float16)
        embts.append(np.ascontiguousarray(et))
        embns.append(
            np.ascontiguousarray(
                chunk.reshape(128, NUM_FIELDS * EMB_DIM).astype(EMBN_NP)
            )
        )
    return embts, embns


def build_in_maps(feature_emb: np.ndarray, W: np.ndarray):
    wt_chunks = _pack_wt(np.asarray(W))
    emb = np.asarray(feature_emb, dtype=np.float32)
    in_maps = []
    for i in range(N_CORES):
        embts, embns = _pack_core_inputs(emb[i * B_CORE : (i + 1) * B_CORE])
        m = {}
        for c in range(N_BCHUNK):
            m[f"embt{c}"] = embts[c]
            m[f"embn{c}"] = embns[c]
        for k, w in enumerate(wt_chunks):
            m[f"wt{k}"] = w
        in_maps.append(m)
    return in_maps


def run(feature_emb: np.ndarray, W: np.ndarray, trace: bool = False, tmpdir=None):
    """Returns (out [2048, 496, 64] fp32, BassKernelResults)."""
    nc = _get_program()
    in_maps = build_in_maps(feature_emb, W)
    res = bass_utils.run_bass_kernel_spmd(
        nc, in_maps, core_ids=list(range(N_CORES)), trace=trace, tmpdir=tmpdir
    )
    out = np.concatenate(
        [res.results[i]["out"] for i in range(N_CORES)], axis=0
    ).astype(np.float32)
    return out, res


def kernel(feature_emb: np.ndarray, W: np.ndarray) -> np.ndarray:
    out, _ = run(feature_emb, W)
    return out
